# revision 21
# baseline (speedup 1.0000x reference)
"""DCRNN kernel for Trainium2 (8 NeuronCores, data-parallel over batch).

Model (per time step t, 6 steps):
    z  = relu([x_t, h] @ fc_w.T + fc_b)          # [b, n, 128]
    zd = einsum('nm,bmh->bnh', adj, z)           # graph diffusion
    GRU(zd, h) -> h                              # gated update
Readout: y = h @ out_w.T + out_b                 # [b, n, 714]

Layout on each core (batch shard of 8):
  - token axis = b*768 + n  (n padded 714->768), TOKP = 6144 tokens
  - state kept feature-major in SBUF: h[128 hid, TOKP] bf16
  - matmuls in bf16 with fp32 PSUM accumulation
  - diffusion needs z token-major; produced via batched xbar DMA
    transposes (adjT rows >= 714 are zero, so padded-token values
    never contribute)
  - program is emitted cross-phase interleaved (GRU(t) with fc(t+1),
    GRU(5) with readout) to keep TensorE dense and HAM-warm
"""
import sys
import types

sys.path.insert(0, "/opt/trn_rl_repo")

import numpy as np
import ml_dtypes
from contextlib import ExitStack

# NTFF profile hook shim: the agent image lacks antenv.axon_hooks; provide it
# so run_bass_kernel_spmd(trace=True) can profile. Harmless when unused.
try:
    import antenv.axon_hooks  # noqa: F401
except ImportError:
    try:
        import trn_agent_boot.trn_boot as _tb

        _m = types.ModuleType("antenv.axon_hooks")
        _hook = _tb._ntff_profile_via_ctypes("/opt/axon/libaxon_pjrt.so")
        _m.get_axon_ntff_profile_hook = lambda: _hook
        _m.set_axon_ntff_profile_hook = lambda h: None
        sys.modules["antenv.axon_hooks"] = _m
    except Exception:
        pass

from concourse import bacc, tile, mybir
from concourse.bass_utils import run_bass_kernel_spmd

F32 = mybir.dt.float32
BF16 = mybir.dt.bfloat16
AF = mybir.ActivationFunctionType
ALU = mybir.AluOpType

B, T, N, D, HID = 64, 6, 714, 16, 128
CORES = 8
BL = B // CORES            # batch per core
NP = 768                   # padded graph size (6*128)
TOKP = BL * NP             # 6144 padded tokens per core
FBLK = 512                 # token block for fc/GRU matmul streams
NBLK = TOKP // FBLK        # 12
NPAIR = NBLK // 2          # 6 pairs of blocks
NCH = NP // 128            # 6 m-chunks per batch item
NH = 357                   # half of the 714 output columns
TGRP = 1536                # tokens per transpose group (2 batch items)

_NC_CACHE = {}


def _build_program():
    if "nc" in _NC_CACHE:
        return _NC_CACHE["nc"]

    nc = bacc.Bacc(
        "TRN2",
        target_bir_lowering=False,
        debug=False,
        enable_asserts=True,
        num_devices=CORES,
    )

    xT_d = nc.declare_dram_parameter("xT", [T, D, TOKP], BF16, isOutput=False)
    adjT_d = nc.declare_dram_parameter("adjT", [NP, N], BF16, isOutput=False)
    fcwx_d = nc.declare_dram_parameter("fc_wxT", [D, HID], BF16, isOutput=False)
    fcwh_d = nc.declare_dram_parameter("fc_whT", [HID, HID], BF16, isOutput=False)
    wih_d = nc.declare_dram_parameter("w_ihT", [HID, 3 * HID], BF16, isOutput=False)
    whh_d = nc.declare_dram_parameter("w_hhT", [HID, 3 * HID], BF16, isOutput=False)
    fcb_d = nc.declare_dram_parameter("fc_b", [HID, 1], F32, isOutput=False)
    br_d = nc.declare_dram_parameter("b_r", [HID, 1], F32, isOutput=False)
    bzn_d = nc.declare_dram_parameter("b_zn", [HID, 1], F32, isOutput=False)
    bin_d = nc.declare_dram_parameter("b_in", [HID, 1], F32, isOutput=False)
    bhn_d = nc.declare_dram_parameter("b_hn", [HID, 1], F32, isOutput=False)
    ident_d = nc.declare_dram_parameter("ident", [128, 128], BF16, isOutput=False)
    outw_d = nc.declare_dram_parameter("out_wT", [HID, N], BF16, isOutput=False)
    outbbc_d = nc.declare_dram_parameter("out_b_bc", [128, N], F32, isOutput=False)
    outbrow_d = nc.declare_dram_parameter("out_b_row", [1, N], BF16, isOutput=False)
    y_d = nc.declare_dram_parameter("y", [BL, N, N], F32, isOutput=True)

    with tile.TileContext(nc) as tc, ExitStack() as ctx:
        cst = ctx.enter_context(tc.tile_pool(name="cst", bufs=1))
        st = ctx.enter_context(tc.tile_pool(name="st", bufs=1))
        xt_p = ctx.enter_context(tc.tile_pool(name="xt_p", bufs=2))
        gb = ctx.enter_context(tc.tile_pool(name="gb", bufs=6))
        gb2 = ctx.enter_context(tc.tile_pool(name="gb2", bufs=4))
        ysb_p = ctx.enter_context(tc.tile_pool(name="ysb", bufs=6))
        ps = ctx.enter_context(tc.tile_pool(name="ps", bufs=8, space="PSUM"))

        # warm the ACT function tables before any real dependency exists
        dummy = cst.tile([1, 16], F32, tag="dummy")
        nc.scalar.activation(dummy[:], dummy[:], AF.Sigmoid)
        nc.scalar.activation(dummy[:], dummy[:], AF.Copy)

        # ---- constants in (fc(0)-critical tensors first) ----
        fc_wxT = cst.tile([D, HID], BF16, tag="fc_wxT")
        nc.sync.dma_start(fc_wxT[:], fcwx_d[:])
        fc_whT = cst.tile([HID, HID], BF16, tag="fc_whT")
        nc.sync.dma_start(fc_whT[:], fcwh_d[:])
        fc_b = cst.tile([HID, 1], F32, tag="fc_b")
        nc.sync.dma_start(fc_b[:], fcb_d[:])
        adjT = []
        for k in range(NCH):
            a = cst.tile([128, N], BF16, tag=f"adjT{k}", name=f"adjT{k}")
            nc.scalar.dma_start(a[:], adjT_d[128 * k : 128 * (k + 1), :])
            adjT.append(a)
        w_ihT = cst.tile([HID, 3 * HID], BF16, tag="w_ihT")
        nc.scalar.dma_start(w_ihT[:], wih_d[:])
        w_hhT = cst.tile([HID, 3 * HID], BF16, tag="w_hhT")
        nc.scalar.dma_start(w_hhT[:], whh_d[:])
        ident = cst.tile([128, 128], BF16, tag="ident")
        nc.scalar.dma_start(ident[:], ident_d[:])
        out_wT = cst.tile([HID, N], BF16, tag="out_wT")
        nc.scalar.dma_start(out_wT[:], outw_d[:])
        out_b_bc = cst.tile([128, N], F32, tag="out_b_bc")
        nc.scalar.dma_start(out_b_bc[:], outbbc_d[:])
        out_b_row = cst.tile([1, N], BF16, tag="out_b_row")
        nc.scalar.dma_start(out_b_row[:], outbrow_d[:])
        ones_col = cst.tile([1, 128], BF16, tag="ones_col")
        nc.gpsimd.memset(ones_col[:], 1.0)
        b_r = cst.tile([HID, 1], F32, tag="b_r")
        nc.scalar.dma_start(b_r[:], br_d[:])
        b_zn = cst.tile([HID, 1], F32, tag="b_zn")
        nc.scalar.dma_start(b_zn[:], bzn_d[:])
        b_in = cst.tile([HID, 1], F32, tag="b_in")
        nc.scalar.dma_start(b_in[:], bin_d[:])
        b_hn = cst.tile([HID, 1], F32, tag="b_hn")
        nc.scalar.dma_start(b_hn[:], bhn_d[:])

        # ---- state ----
        h0 = st.tile([HID, TOKP], BF16, tag="h0")
        h1 = st.tile([HID, TOKP], BF16, tag="h1")
        z_fm = st.tile([HID, TOKP], BF16, tag="z_fm")
        zd0 = st.tile([HID, TOKP], BF16, tag="zd0")
        zd1 = st.tile([HID, TOKP], BF16, tag="zd1")
        zt_all = st.tile([128, BL * NCH, 128], BF16, tag="zt_all")
        # h0 must be zero (initial state); h1 is fully written before read.
        # zd only needs its pad columns (tokens 714..767 of each window)
        # zeroed once — diffusion never writes them, GRU reads them.
        nc.vector.memset(h0[:], 0.0)
        nc.gpsimd.memset(
            zd0.rearrange("p (b n) -> p b n", b=BL)[:, :, N:NP], 0.0)
        nc.gpsimd.memset(
            zd1.rearrange("p (b n) -> p b n", b=BL)[:, :, N:NP], 0.0)
        hbuf = [h0, h1]
        zdbuf = [zd0, zd1]

        def load_xt(t):
            xt = xt_p.tile([D, TOKP], BF16, tag="xt", name="xt")
            nc.sync.dma_start(xt[:], xT_d[t])
            return xt

        def fc_block(t, i, xt, hc):
            """z[:, blk] = relu(Wx@xt + Wh@h + fc_b)"""
            s0, s1 = FBLK * i, FBLK * (i + 1)
            psz = ps.tile([128, FBLK], F32, tag="blk", name="psz")
            nc.tensor.matmul(psz[:], fc_wxT[:], xt[:, s0:s1],
                             start=True, stop=False)
            nc.tensor.matmul(psz[:], fc_whT[:], hc[:, s0:s1],
                             start=False, stop=True)
            nc.scalar.activation(z_fm[:, s0:s1], psz[:], AF.Relu,
                                 bias=fc_b[:])

        def transpose_group(j):
            """xbar-transpose tokens [1536j, 1536(j+1)) of z into zt_all."""
            nc.sync.dma_start(
                zt_all[:, 12 * j : 12 * (j + 1), :],
                z_fm[:, TGRP * j : TGRP * (j + 1)],
                transpose=True)

        def diffusion_b(b, zdn):
            """zd[:, b-window] = z_b.T-chunks @ adjT  (contract over m)."""
            base = NP * b
            psa = ps.tile([128, FBLK], F32, tag="blk", name="psa")
            psb = ps.tile([128, FBLK], F32, tag="blk", name="psb")
            for k in range(NCH):
                zt = zt_all[:, NCH * b + k, :]
                nc.tensor.matmul(psa[:, 0:NH], zt, adjT[k][:, 0:NH],
                                 start=(k == 0), stop=(k == NCH - 1))
                nc.tensor.matmul(psb[:, 0:NH], zt, adjT[k][:, NH:N],
                                 start=(k == 0), stop=(k == NCH - 1))
            nc.vector.tensor_copy(zdn[:, base : base + NH], psa[:, 0:NH])
            nc.vector.tensor_copy(zdn[:, base + NH : base + N],
                                  psb[:, 0:NH])

        def gru_pair_a(p, hc, zdc):
            """GRU stage A for blocks 2p, 2p+1: r/u/hn matmuls + r/u1/t1."""
            u2 = gb2.tile([128, 2 * FBLK], BF16, tag="u2", name="u2")
            t1s = []
            for half, i in enumerate((2 * p, 2 * p + 1)):
                s0, s1 = FBLK * i, FBLK * (i + 1)
                o0, o1 = FBLK * half, FBLK * (half + 1)
                ps_hn = ps.tile([128, FBLK], F32, tag="blk", name="ps_hn")
                nc.tensor.matmul(ps_hn[:], w_hhT[:, 256:384], hc[:, s0:s1],
                                 start=True, stop=True)
                ps_r = ps.tile([128, FBLK], F32, tag="blk", name="ps_r")
                nc.tensor.matmul(ps_r[:], w_ihT[:, 0:128], zdc[:, s0:s1],
                                 start=True, stop=False)
                nc.tensor.matmul(ps_r[:], w_hhT[:, 0:128], hc[:, s0:s1],
                                 start=False, stop=True)
                ps_u = ps.tile([128, FBLK], F32, tag="blk", name="ps_u")
                nc.tensor.matmul(ps_u[:], w_ihT[:, 128:256], zdc[:, s0:s1],
                                 start=True, stop=False)
                nc.tensor.matmul(ps_u[:], w_hhT[:, 128:256], hc[:, s0:s1],
                                 start=False, stop=True)

                r = gb.tile([128, FBLK], BF16, tag="r", name="r")
                nc.scalar.activation(r[:], ps_r[:], AF.Sigmoid, bias=b_r[:])
                nc.scalar.activation(u2[:, o0:o1], ps_u[:], AF.Sigmoid,
                                     bias=b_zn[:], scale=-1.0)
                t1 = gb.tile([128, FBLK], BF16, tag="t1", name="t1")
                nc.vector.scalar_tensor_tensor(t1[:], ps_hn[:], b_hn[:], r[:],
                                               ALU.add, ALU.mult)
                t1s.append(t1)
            return u2, t1s

        def gru_pair_b(p, hc, hn, zdc, ab):
            """GRU stage B for blocks 2p, 2p+1: the n-gate matmul accumulates
            t1 via an identity matmul (saving a DVE pass), then
            h' = h + (1-u)*(tanh(i_n + b_in + t1) - h)."""
            u2, t1s = ab
            c2 = gb2.tile([128, 2 * FBLK], BF16, tag="c2", name="c2")
            for half, i in enumerate((2 * p, 2 * p + 1)):
                s0, s1 = FBLK * i, FBLK * (i + 1)
                o0, o1 = FBLK * half, FBLK * (half + 1)
                ps_in = ps.tile([128, FBLK], F32, tag="blk", name="ps_in")
                nc.tensor.matmul(ps_in[:], w_ihT[:, 256:384], zdc[:, s0:s1],
                                 start=True, stop=False)
                nc.tensor.matmul(ps_in[:], ident[:], t1s[half][:],
                                 start=False, stop=True)
                nc.scalar.activation(c2[:, o0:o1], ps_in[:], AF.Tanh,
                                     bias=b_in[:])
            s0, s1 = 2 * FBLK * p, 2 * FBLK * (p + 1)
            d2 = gb2.tile([128, 2 * FBLK], BF16, tag="d2", name="d2")
            nc.vector.tensor_tensor(d2[:], c2[:], hc[:, s0:s1], ALU.subtract)
            e2 = gb2.tile([128, 2 * FBLK], BF16, tag="e2", name="e2")
            nc.vector.tensor_tensor(e2[:], u2[:], d2[:], ALU.mult)
            nc.vector.tensor_tensor(hn[:, s0:s1], hc[:, s0:s1], e2[:], ALU.add)

        def readout_chunk(c, hF):
            b, k = divmod(c, NCH)
            rows = 128 if k < NCH - 1 else N - 128 * (NCH - 1)
            tk0 = NP * b + 128 * k
            hch = hF[:, tk0:tk0 + 128]
            psa = ps.tile([128, FBLK], F32, tag="blk", name="pya")
            psb = ps.tile([128, FBLK], F32, tag="blk", name="pyb")
            y_sb = ysb_p.tile([128, N], F32, tag="y_sb", name="y_sb")
            if c % 2 == 0:
                # bias via DVE scalar_tensor_tensor; store from sync queue
                nc.tensor.matmul(psa[:, 0:NH], hch, out_wT[:, 0:NH],
                                 start=True, stop=True)
                nc.tensor.matmul(psb[:, 0:NH], hch, out_wT[:, NH:N],
                                 start=True, stop=True)
                nc.vector.scalar_tensor_tensor(
                    y_sb[:, 0:NH], psa[:, 0:NH], 0.0, out_b_bc[:, 0:NH],
                    ALU.bypass, ALU.add)
                nc.vector.scalar_tensor_tensor(
                    y_sb[:, NH:N], psb[:, 0:NH], 0.0, out_b_bc[:, NH:N],
                    ALU.bypass, ALU.add)
                nc.sync.dma_start(y_d[b, 128 * k : 128 * k + rows, :],
                                  y_sb[0:rows, :])
            else:
                # bias via rank-1 matmul + ACT copies; store from the scalar
                # queue right behind its own copies (tiny queue wait)
                nc.tensor.matmul(psa[:, 0:NH], ones_col[:],
                                 out_b_row[:, 0:NH], start=True, stop=False)
                nc.tensor.matmul(psa[:, 0:NH], hch, out_wT[:, 0:NH],
                                 start=False, stop=True)
                nc.tensor.matmul(psb[:, 0:NH], ones_col[:],
                                 out_b_row[:, NH:N], start=True, stop=False)
                nc.tensor.matmul(psb[:, 0:NH], hch, out_wT[:, NH:N],
                                 start=False, stop=True)
                nc.scalar.activation(y_sb[:, 0:NH], psa[:, 0:NH], AF.Copy)
                nc.scalar.activation(y_sb[:, NH:N], psb[:, 0:NH], AF.Copy)
                nc.scalar.dma_start(y_d[b, 128 * k : 128 * k + rows, :],
                                    y_sb[0:rows, :])

        # ---- prologue: step 0 fc + transposes + diffusion ----
        xt_cur = load_xt(0)
        for i in range(NBLK):
            fc_block(0, i, xt_cur, hbuf[0])
        for j in range(4):
            transpose_group(j)
        for b in range(BL):
            diffusion_b(b, zdbuf[0])

        # ---- main loop ----
        # GRU(t) pairs are the backbone; work for step t+1 (fc, transposes,
        # diffusion b0..b3) rides 2+ pairs behind its dependencies, and
        # diffusion(t) b4..b7 was deferred into this step's own pair loop
        # (slack >= 2 pairs everywhere so TensorE never head-of-line blocks).
        for t in range(T):
            hc, hn = hbuf[t % 2], hbuf[(t + 1) % 2]
            zdc = zdbuf[t % 2]
            zdn = zdbuf[(t + 1) % 2]
            if t + 1 < T:
                xt_nxt = load_xt(t + 1)
            ab = [None] * NPAIR
            for p in range(NPAIR):
                ab[p] = gru_pair_a(p, hc, zdc)
                if p >= 1:
                    gru_pair_b(p - 1, hc, hn, zdc, ab[p - 1])
                if t > 0 and p == 0:
                    diffusion_b(4, zdc)
                    diffusion_b(5, zdc)
                if t > 0 and p == 1:
                    diffusion_b(6, zdc)
                    diffusion_b(7, zdc)
                if t + 1 < T:
                    if p >= 2:
                        fc_block(t + 1, 2 * (p - 2), xt_nxt, hn)
                        fc_block(t + 1, 2 * (p - 2) + 1, xt_nxt, hn)
                    if p == 3:
                        transpose_group(0)
                    if p == 4:
                        transpose_group(1)
                    if p == 5:
                        fc_block(t + 1, 6, xt_nxt, hn)
                        fc_block(t + 1, 7, xt_nxt, hn)
                        diffusion_b(0, zdn)
                        diffusion_b(1, zdn)
                else:
                    if p >= 1:
                        for c in range(8 * (p - 1), 8 * p):
                            readout_chunk(c, hn)
            if t + 1 < T:
                for i in (8, 9):
                    fc_block(t + 1, i, xt_nxt, hn)
                transpose_group(2)
                gru_pair_b(NPAIR - 1, hc, hn, zdc, ab[NPAIR - 1])
                diffusion_b(2, zdn)
                diffusion_b(3, zdn)
                for i in (10, 11):
                    fc_block(t + 1, i, xt_nxt, hn)
                transpose_group(3)
            else:
                gru_pair_b(NPAIR - 1, hc, hn, zdc, ab[NPAIR - 1])
                for c in range(40, 48):
                    readout_chunk(c, hn)
            xt_cur = xt_nxt if t + 1 < T else None

    nc.compile()
    _NC_CACHE["nc"] = nc
    return nc


def _prep_core_inputs(x_core, shared):
    m = dict(shared)
    xp = np.zeros((T, D, BL, NP), dtype=np.float32)
    xp[:, :, :, :N] = x_core.transpose(1, 3, 0, 2)
    m["xT"] = xp.reshape(T, D, TOKP).astype(ml_dtypes.bfloat16)
    return m


def run(inputs, trace=False):
    x = np.asarray(inputs["x"], np.float32)
    adj = np.asarray(inputs["adj"], np.float32)
    fc_w = np.asarray(inputs["fc_w"], np.float32)
    fc_b = np.asarray(inputs["fc_b"], np.float32)
    w_ih = np.asarray(inputs["w_ih"], np.float32)
    w_hh = np.asarray(inputs["w_hh"], np.float32)
    b_ih = np.asarray(inputs["b_ih"], np.float32)
    b_hh = np.asarray(inputs["b_hh"], np.float32)
    out_w = np.asarray(inputs["out_w"], np.float32)
    out_b = np.asarray(inputs["out_b"], np.float32)

    adjT = np.zeros((NP, N), np.float32)
    adjT[:N, :] = adj.T
    shared = {
        "adjT": adjT.astype(ml_dtypes.bfloat16),
        "fc_wxT": np.ascontiguousarray(fc_w[:, :D].T).astype(ml_dtypes.bfloat16),
        "fc_whT": np.ascontiguousarray(fc_w[:, D:].T).astype(ml_dtypes.bfloat16),
        "w_ihT": np.ascontiguousarray(w_ih.T).astype(ml_dtypes.bfloat16),
        "w_hhT": np.ascontiguousarray(w_hh.T).astype(ml_dtypes.bfloat16),
        "fc_b": fc_b.reshape(HID, 1).copy(),
        "b_r": (b_ih[0:128] + b_hh[0:128]).reshape(HID, 1),
        "b_zn": (-(b_ih[128:256] + b_hh[128:256])).reshape(HID, 1),
        "b_in": b_ih[256:384].reshape(HID, 1).copy(),
        "b_hn": b_hh[256:384].reshape(HID, 1).copy(),
        "ident": np.eye(128, dtype=np.float32).astype(ml_dtypes.bfloat16),
        "out_wT": np.ascontiguousarray(out_w.T).astype(ml_dtypes.bfloat16),
        "out_b_bc": np.ascontiguousarray(
            np.broadcast_to(out_b, (128, N))).astype(np.float32),
        "out_b_row": out_b.reshape(1, N).astype(ml_dtypes.bfloat16),
    }

    nc = _build_program()
    in_maps = [_prep_core_inputs(x[BL * i : BL * (i + 1)], shared)
               for i in range(CORES)]
    res = run_bass_kernel_spmd(nc, in_maps, list(range(CORES)), trace=trace)
    y = np.concatenate([res.results[i]["y"] for i in range(CORES)], axis=0)
    return y, res


def kernel(**inputs) -> np.ndarray:
    y, _ = run(inputs, trace=False)
    return y


# revision 22
# speedup vs baseline: 1.0384x; 1.0384x over previous
"""DCRNN kernel for Trainium2 (8 NeuronCores, data-parallel over batch).

Model (per time step t, 6 steps):
    z  = relu([x_t, h] @ fc_w.T + fc_b)          # [b, n, 128]
    zd = einsum('nm,bmh->bnh', adj, z)           # graph diffusion
    GRU(zd, h) -> h                              # gated update
Readout: y = h @ out_w.T + out_b                 # [b, n, 714]

Layout on each core (batch shard of 8):
  - token axis = b*768 + n  (n padded 714->768), TOKP = 6144 tokens
  - state kept feature-major in SBUF: h[128 hid, TOKP] bf16
  - matmuls in bf16 with fp32 PSUM accumulation
  - diffusion needs z token-major; produced via batched xbar DMA
    transposes (adjT rows >= 714 are zero, so padded-token values
    never contribute)
  - program is emitted cross-phase interleaved (GRU(t) with fc(t+1),
    GRU(5) with readout) to keep TensorE dense and HAM-warm
"""
import sys
import types

sys.path.insert(0, "/opt/trn_rl_repo")

import numpy as np
import ml_dtypes
from contextlib import ExitStack

# NTFF profile hook shim: the agent image lacks antenv.axon_hooks; provide it
# so run_bass_kernel_spmd(trace=True) can profile. Harmless when unused.
try:
    import antenv.axon_hooks  # noqa: F401
except ImportError:
    try:
        import trn_agent_boot.trn_boot as _tb

        _m = types.ModuleType("antenv.axon_hooks")
        _hook = _tb._ntff_profile_via_ctypes("/opt/axon/libaxon_pjrt.so")
        _m.get_axon_ntff_profile_hook = lambda: _hook
        _m.set_axon_ntff_profile_hook = lambda h: None
        sys.modules["antenv.axon_hooks"] = _m
    except Exception:
        pass

from concourse import bacc, tile, mybir
from concourse.bass_utils import run_bass_kernel_spmd

F32 = mybir.dt.float32
BF16 = mybir.dt.bfloat16
AF = mybir.ActivationFunctionType
ALU = mybir.AluOpType

B, T, N, D, HID = 64, 6, 714, 16, 128
CORES = 8
BL = B // CORES            # batch per core
NP = 768                   # padded graph size (6*128)
TOKP = BL * NP             # 6144 padded tokens per core
FBLK = 512                 # token block for fc/GRU matmul streams
NBLK = TOKP // FBLK        # 12
NPAIR = NBLK // 2          # 6 pairs of blocks
NCH = NP // 128            # 6 m-chunks per batch item
NH = 357                   # half of the 714 output columns
TGRP = 1536                # tokens per transpose group (2 batch items)

_NC_CACHE = {}


def _build_program():
    if "nc" in _NC_CACHE:
        return _NC_CACHE["nc"]

    nc = bacc.Bacc(
        "TRN2",
        target_bir_lowering=False,
        debug=False,
        enable_asserts=True,
        num_devices=CORES,
    )

    xT_d = nc.declare_dram_parameter("xT", [T, D, TOKP], BF16, isOutput=False)
    adjT_d = nc.declare_dram_parameter("adjT", [NP, N], BF16, isOutput=False)
    fcwx_d = nc.declare_dram_parameter("fc_wxT", [D, HID], BF16, isOutput=False)
    fcwh_d = nc.declare_dram_parameter("fc_whT", [HID, HID], BF16, isOutput=False)
    wih_d = nc.declare_dram_parameter("w_ihT", [HID, 3 * HID], BF16, isOutput=False)
    whh_d = nc.declare_dram_parameter("w_hhT", [HID, 3 * HID], BF16, isOutput=False)
    fcb_d = nc.declare_dram_parameter("fc_b", [HID, 1], F32, isOutput=False)
    br_d = nc.declare_dram_parameter("b_r", [HID, 1], F32, isOutput=False)
    bzn_d = nc.declare_dram_parameter("b_zn", [HID, 1], F32, isOutput=False)
    bin_d = nc.declare_dram_parameter("b_in", [HID, 1], F32, isOutput=False)
    bhn_d = nc.declare_dram_parameter("b_hn", [HID, 1], F32, isOutput=False)
    ident_d = nc.declare_dram_parameter("ident", [128, 128], BF16, isOutput=False)
    outw_d = nc.declare_dram_parameter("out_wT", [HID, N], BF16, isOutput=False)
    outbbc_d = nc.declare_dram_parameter("out_b_bc", [128, N], F32, isOutput=False)
    outbrow_d = nc.declare_dram_parameter("out_b_row", [1, N], BF16, isOutput=False)
    y_d = nc.declare_dram_parameter("y", [BL, N, N], F32, isOutput=True)

    with tile.TileContext(nc) as tc, ExitStack() as ctx:
        cst = ctx.enter_context(tc.tile_pool(name="cst", bufs=1))
        st = ctx.enter_context(tc.tile_pool(name="st", bufs=1))
        xt_p = ctx.enter_context(tc.tile_pool(name="xt_p", bufs=2))
        gb = ctx.enter_context(tc.tile_pool(name="gb", bufs=6))
        gb2 = ctx.enter_context(tc.tile_pool(name="gb2", bufs=4))
        ysb_p = ctx.enter_context(tc.tile_pool(name="ysb", bufs=6))
        ps = ctx.enter_context(tc.tile_pool(name="ps", bufs=8, space="PSUM"))

        # warm the ACT function tables before any real dependency exists
        dummy = cst.tile([1, 16], F32, tag="dummy")
        nc.scalar.activation(dummy[:], dummy[:], AF.Sigmoid)
        nc.scalar.activation(dummy[:], dummy[:], AF.Copy)

        # ---- constants in (fc(0)-critical tensors first) ----
        fc_wxT = cst.tile([D, HID], BF16, tag="fc_wxT")
        nc.sync.dma_start(fc_wxT[:], fcwx_d[:])
        fc_whT = cst.tile([HID, HID], BF16, tag="fc_whT")
        nc.sync.dma_start(fc_whT[:], fcwh_d[:])
        fc_b = cst.tile([HID, 1], F32, tag="fc_b")
        nc.sync.dma_start(fc_b[:], fcb_d[:])
        adjT = []
        for k in range(NCH):
            a = cst.tile([128, N], BF16, tag=f"adjT{k}", name=f"adjT{k}")
            nc.scalar.dma_start(a[:], adjT_d[128 * k : 128 * (k + 1), :])
            adjT.append(a)
        w_ihT = cst.tile([HID, 3 * HID], BF16, tag="w_ihT")
        nc.scalar.dma_start(w_ihT[:], wih_d[:])
        w_hhT = cst.tile([HID, 3 * HID], BF16, tag="w_hhT")
        nc.scalar.dma_start(w_hhT[:], whh_d[:])
        ident = cst.tile([128, 128], BF16, tag="ident")
        nc.scalar.dma_start(ident[:], ident_d[:])
        out_wT = cst.tile([HID, N], BF16, tag="out_wT")
        nc.scalar.dma_start(out_wT[:], outw_d[:])
        out_b_bc = cst.tile([128, N], F32, tag="out_b_bc")
        nc.scalar.dma_start(out_b_bc[:], outbbc_d[:])
        out_b_row = cst.tile([1, N], BF16, tag="out_b_row")
        nc.scalar.dma_start(out_b_row[:], outbrow_d[:])
        ones_col = cst.tile([1, 128], BF16, tag="ones_col")
        nc.gpsimd.memset(ones_col[:], 1.0)
        b_r = cst.tile([HID, 1], F32, tag="b_r")
        nc.scalar.dma_start(b_r[:], br_d[:])
        b_zn = cst.tile([HID, 1], F32, tag="b_zn")
        nc.scalar.dma_start(b_zn[:], bzn_d[:])
        b_in = cst.tile([HID, 1], F32, tag="b_in")
        nc.scalar.dma_start(b_in[:], bin_d[:])
        b_hn = cst.tile([HID, 1], F32, tag="b_hn")
        nc.scalar.dma_start(b_hn[:], bhn_d[:])

        # ---- state ----
        h0 = st.tile([HID, TOKP], BF16, tag="h0")
        h1 = st.tile([HID, TOKP], BF16, tag="h1")
        z_fm = st.tile([HID, TOKP], BF16, tag="z_fm")
        zd0 = st.tile([HID, TOKP], BF16, tag="zd0")
        zd1 = st.tile([HID, TOKP], BF16, tag="zd1")
        zt_all = st.tile([128, BL * NCH, 128], BF16, tag="zt_all")
        # h0 must be zero (initial state); h1 is fully written before read.
        # zd only needs its pad columns (tokens 714..767 of each window)
        # zeroed once — diffusion never writes them, GRU reads them.
        nc.vector.memset(h0[:], 0.0)
        nc.gpsimd.memset(
            zd0.rearrange("p (b n) -> p b n", b=BL)[:, :, N:NP], 0.0)
        nc.gpsimd.memset(
            zd1.rearrange("p (b n) -> p b n", b=BL)[:, :, N:NP], 0.0)
        hbuf = [h0, h1]
        zdbuf = [zd0, zd1]

        def load_xt(t):
            xt = xt_p.tile([D, TOKP], BF16, tag="xt", name="xt")
            nc.sync.dma_start(xt[:], xT_d[t])
            return xt

        def fc_block(t, i, xt, hc):
            """z[:, blk] = relu(Wx@xt + Wh@h + fc_b)"""
            s0, s1 = FBLK * i, FBLK * (i + 1)
            psz = ps.tile([128, FBLK], F32, tag="blk", name="psz")
            nc.tensor.matmul(psz[:], fc_wxT[:], xt[:, s0:s1],
                             start=True, stop=False)
            nc.tensor.matmul(psz[:], fc_whT[:], hc[:, s0:s1],
                             start=False, stop=True)
            nc.scalar.activation(z_fm[:, s0:s1], psz[:], AF.Relu,
                                 bias=fc_b[:])

        def transpose_group(j):
            """xbar-transpose tokens [1536j, 1536(j+1)) of z into zt_all."""
            nc.sync.dma_start(
                zt_all[:, 12 * j : 12 * (j + 1), :],
                z_fm[:, TGRP * j : TGRP * (j + 1)],
                transpose=True)

        def diffusion_b(b, zdn):
            """zd[:, b-window] = z_b.T-chunks @ adjT  (contract over m)."""
            base = NP * b
            psa = ps.tile([128, FBLK], F32, tag="blk", name="psa")
            psb = ps.tile([128, FBLK], F32, tag="blk", name="psb")
            for k in range(NCH):
                zt = zt_all[:, NCH * b + k, :]
                nc.tensor.matmul(psa[:, 0:NH], zt, adjT[k][:, 0:NH],
                                 start=(k == 0), stop=(k == NCH - 1))
                nc.tensor.matmul(psb[:, 0:NH], zt, adjT[k][:, NH:N],
                                 start=(k == 0), stop=(k == NCH - 1))
            if b % 2 == 0:
                nc.scalar.activation(zdn[:, base : base + NH],
                                     psa[:, 0:NH], AF.Copy)
                nc.scalar.activation(zdn[:, base + NH : base + N],
                                     psb[:, 0:NH], AF.Copy)
            else:
                nc.vector.tensor_copy(zdn[:, base : base + NH], psa[:, 0:NH])
                nc.vector.tensor_copy(zdn[:, base + NH : base + N],
                                      psb[:, 0:NH])

        def gru_pair_a(p, hc, zdc):
            """GRU stage A for blocks 2p, 2p+1: r/u/hn matmuls + r/u1/t1."""
            u2 = gb2.tile([128, 2 * FBLK], BF16, tag="u2", name="u2")
            t1s = []
            for half, i in enumerate((2 * p, 2 * p + 1)):
                s0, s1 = FBLK * i, FBLK * (i + 1)
                o0, o1 = FBLK * half, FBLK * (half + 1)
                ps_hn = ps.tile([128, FBLK], F32, tag="blk", name="ps_hn")
                nc.tensor.matmul(ps_hn[:], w_hhT[:, 256:384], hc[:, s0:s1],
                                 start=True, stop=True)
                ps_r = ps.tile([128, FBLK], F32, tag="blk", name="ps_r")
                nc.tensor.matmul(ps_r[:], w_ihT[:, 0:128], zdc[:, s0:s1],
                                 start=True, stop=False)
                nc.tensor.matmul(ps_r[:], w_hhT[:, 0:128], hc[:, s0:s1],
                                 start=False, stop=True)
                ps_u = ps.tile([128, FBLK], F32, tag="blk", name="ps_u")
                nc.tensor.matmul(ps_u[:], w_ihT[:, 128:256], zdc[:, s0:s1],
                                 start=True, stop=False)
                nc.tensor.matmul(ps_u[:], w_hhT[:, 128:256], hc[:, s0:s1],
                                 start=False, stop=True)

                r = gb.tile([128, FBLK], BF16, tag="r", name="r")
                nc.scalar.activation(r[:], ps_r[:], AF.Sigmoid, bias=b_r[:])
                nc.scalar.activation(u2[:, o0:o1], ps_u[:], AF.Sigmoid,
                                     bias=b_zn[:], scale=-1.0)
                t1 = gb.tile([128, FBLK], BF16, tag="t1", name="t1")
                nc.vector.scalar_tensor_tensor(t1[:], ps_hn[:], b_hn[:], r[:],
                                               ALU.add, ALU.mult)
                t1s.append(t1)
            return u2, t1s

        def gru_pair_b(p, hc, hn, zdc, ab):
            """GRU stage B for blocks 2p, 2p+1:
            h' = h + (1-u)*(tanh(i_n + b_in + t1) - h)."""
            u2, t1s = ab
            sg2 = gb2.tile([128, 2 * FBLK], BF16, tag="sg2", name="sg2")
            for half, i in enumerate((2 * p, 2 * p + 1)):
                s0, s1 = FBLK * i, FBLK * (i + 1)
                o0, o1 = FBLK * half, FBLK * (half + 1)
                ps_in = ps.tile([128, FBLK], F32, tag="blk", name="ps_in")
                nc.tensor.matmul(ps_in[:], w_ihT[:, 256:384], zdc[:, s0:s1],
                                 start=True, stop=True)
                nc.vector.scalar_tensor_tensor(sg2[:, o0:o1], ps_in[:],
                                               b_in[:], t1s[half][:],
                                               ALU.add, ALU.add)
            s0, s1 = 2 * FBLK * p, 2 * FBLK * (p + 1)
            c2 = gb2.tile([128, 2 * FBLK], BF16, tag="c2", name="c2")
            nc.scalar.activation(c2[:], sg2[:], AF.Tanh)
            d2 = gb2.tile([128, 2 * FBLK], BF16, tag="d2", name="d2")
            nc.vector.tensor_tensor(d2[:], c2[:], hc[:, s0:s1], ALU.subtract)
            e2 = gb2.tile([128, 2 * FBLK], BF16, tag="e2", name="e2")
            nc.vector.tensor_tensor(e2[:], u2[:], d2[:], ALU.mult)
            nc.vector.tensor_tensor(hn[:, s0:s1], hc[:, s0:s1], e2[:], ALU.add)

        def readout_chunk(c, hF):
            b, k = divmod(c, NCH)
            rows = 128 if k < NCH - 1 else N - 128 * (NCH - 1)
            tk0 = NP * b + 128 * k
            hch = hF[:, tk0:tk0 + 128]
            psa = ps.tile([128, FBLK], F32, tag="blk", name="pya")
            psb = ps.tile([128, FBLK], F32, tag="blk", name="pyb")
            y_sb = ysb_p.tile([128, N], F32, tag="y_sb", name="y_sb")
            if c % 2 == 0:
                # bias via DVE scalar_tensor_tensor; store from sync queue
                nc.tensor.matmul(psa[:, 0:NH], hch, out_wT[:, 0:NH],
                                 start=True, stop=True)
                nc.tensor.matmul(psb[:, 0:NH], hch, out_wT[:, NH:N],
                                 start=True, stop=True)
                nc.vector.scalar_tensor_tensor(
                    y_sb[:, 0:NH], psa[:, 0:NH], 0.0, out_b_bc[:, 0:NH],
                    ALU.bypass, ALU.add)
                nc.vector.scalar_tensor_tensor(
                    y_sb[:, NH:N], psb[:, 0:NH], 0.0, out_b_bc[:, NH:N],
                    ALU.bypass, ALU.add)
                nc.sync.dma_start(y_d[b, 128 * k : 128 * k + rows, :],
                                  y_sb[0:rows, :])
            else:
                # bias via rank-1 matmul + ACT copies; store from the scalar
                # queue right behind its own copies (tiny queue wait)
                nc.tensor.matmul(psa[:, 0:NH], ones_col[:],
                                 out_b_row[:, 0:NH], start=True, stop=False)
                nc.tensor.matmul(psa[:, 0:NH], hch, out_wT[:, 0:NH],
                                 start=False, stop=True)
                nc.tensor.matmul(psb[:, 0:NH], ones_col[:],
                                 out_b_row[:, NH:N], start=True, stop=False)
                nc.tensor.matmul(psb[:, 0:NH], hch, out_wT[:, NH:N],
                                 start=False, stop=True)
                nc.scalar.activation(y_sb[:, 0:NH], psa[:, 0:NH], AF.Copy)
                nc.scalar.activation(y_sb[:, NH:N], psb[:, 0:NH], AF.Copy)
                nc.scalar.dma_start(y_d[b, 128 * k : 128 * k + rows, :],
                                    y_sb[0:rows, :])

        # ---- prologue: step 0 fc + transposes + diffusion ----
        xt_cur = load_xt(0)
        for i in range(NBLK):
            fc_block(0, i, xt_cur, hbuf[0])
        for j in range(4):
            transpose_group(j)
        for b in range(BL):
            diffusion_b(b, zdbuf[0])

        # ---- main loop ----
        # GRU(t) pairs are the backbone; work for step t+1 (fc, transposes,
        # diffusion b0..b3) rides 2+ pairs behind its dependencies, and
        # diffusion(t) b4..b7 was deferred into this step's own pair loop
        # (slack >= 2 pairs everywhere so TensorE never head-of-line blocks).
        for t in range(T):
            hc, hn = hbuf[t % 2], hbuf[(t + 1) % 2]
            zdc = zdbuf[t % 2]
            zdn = zdbuf[(t + 1) % 2]
            if t + 1 < T:
                xt_nxt = load_xt(t + 1)
            ab = [None] * NPAIR
            for p in range(NPAIR):
                ab[p] = gru_pair_a(p, hc, zdc)
                if p >= 1:
                    gru_pair_b(p - 1, hc, hn, zdc, ab[p - 1])
                if t > 0 and p == 0:
                    diffusion_b(4, zdc)
                    diffusion_b(5, zdc)
                if t > 0 and p == 1:
                    diffusion_b(6, zdc)
                    diffusion_b(7, zdc)
                if t + 1 < T:
                    if p >= 2:
                        fc_block(t + 1, 2 * (p - 2), xt_nxt, hn)
                        fc_block(t + 1, 2 * (p - 2) + 1, xt_nxt, hn)
                    if p == 3:
                        transpose_group(0)
                    if p == 4:
                        transpose_group(1)
                    if p == 5:
                        fc_block(t + 1, 6, xt_nxt, hn)
                        fc_block(t + 1, 7, xt_nxt, hn)
                        diffusion_b(0, zdn)
                        diffusion_b(1, zdn)
                else:
                    if p >= 2:
                        for c in range(8 * (p - 2), 8 * (p - 1)):
                            readout_chunk(c, hn)
            if t + 1 < T:
                for i in (8, 9):
                    fc_block(t + 1, i, xt_nxt, hn)
                transpose_group(2)
                gru_pair_b(NPAIR - 1, hc, hn, zdc, ab[NPAIR - 1])
                diffusion_b(2, zdn)
                diffusion_b(3, zdn)
                for i in (10, 11):
                    fc_block(t + 1, i, xt_nxt, hn)
                transpose_group(3)
            else:
                gru_pair_b(NPAIR - 1, hc, hn, zdc, ab[NPAIR - 1])
                for c in range(32, 48):
                    readout_chunk(c, hn)
            xt_cur = xt_nxt if t + 1 < T else None

    nc.compile()
    _NC_CACHE["nc"] = nc
    return nc


def _prep_core_inputs(x_core, shared):
    m = dict(shared)
    xp = np.zeros((T, D, BL, NP), dtype=np.float32)
    xp[:, :, :, :N] = x_core.transpose(1, 3, 0, 2)
    m["xT"] = xp.reshape(T, D, TOKP).astype(ml_dtypes.bfloat16)
    return m


def run(inputs, trace=False):
    x = np.asarray(inputs["x"], np.float32)
    adj = np.asarray(inputs["adj"], np.float32)
    fc_w = np.asarray(inputs["fc_w"], np.float32)
    fc_b = np.asarray(inputs["fc_b"], np.float32)
    w_ih = np.asarray(inputs["w_ih"], np.float32)
    w_hh = np.asarray(inputs["w_hh"], np.float32)
    b_ih = np.asarray(inputs["b_ih"], np.float32)
    b_hh = np.asarray(inputs["b_hh"], np.float32)
    out_w = np.asarray(inputs["out_w"], np.float32)
    out_b = np.asarray(inputs["out_b"], np.float32)

    adjT = np.zeros((NP, N), np.float32)
    adjT[:N, :] = adj.T
    shared = {
        "adjT": adjT.astype(ml_dtypes.bfloat16),
        "fc_wxT": np.ascontiguousarray(fc_w[:, :D].T).astype(ml_dtypes.bfloat16),
        "fc_whT": np.ascontiguousarray(fc_w[:, D:].T).astype(ml_dtypes.bfloat16),
        "w_ihT": np.ascontiguousarray(w_ih.T).astype(ml_dtypes.bfloat16),
        "w_hhT": np.ascontiguousarray(w_hh.T).astype(ml_dtypes.bfloat16),
        "fc_b": fc_b.reshape(HID, 1).copy(),
        "b_r": (b_ih[0:128] + b_hh[0:128]).reshape(HID, 1),
        "b_zn": (-(b_ih[128:256] + b_hh[128:256])).reshape(HID, 1),
        "b_in": b_ih[256:384].reshape(HID, 1).copy(),
        "b_hn": b_hh[256:384].reshape(HID, 1).copy(),
        "ident": np.eye(128, dtype=np.float32).astype(ml_dtypes.bfloat16),
        "out_wT": np.ascontiguousarray(out_w.T).astype(ml_dtypes.bfloat16),
        "out_b_bc": np.ascontiguousarray(
            np.broadcast_to(out_b, (128, N))).astype(np.float32),
        "out_b_row": out_b.reshape(1, N).astype(ml_dtypes.bfloat16),
    }

    nc = _build_program()
    in_maps = [_prep_core_inputs(x[BL * i : BL * (i + 1)], shared)
               for i in range(CORES)]
    res = run_bass_kernel_spmd(nc, in_maps, list(range(CORES)), trace=trace)
    y = np.concatenate([res.results[i]["y"] for i in range(CORES)], axis=0)
    return y, res


def kernel(**inputs) -> np.ndarray:
    y, _ = run(inputs, trace=False)
    return y


# revision 23
# speedup vs baseline: 1.0517x; 1.0128x over previous
"""DCRNN kernel for Trainium2 (8 NeuronCores, data-parallel over batch).

Model (per time step t, 6 steps):
    z  = relu([x_t, h] @ fc_w.T + fc_b)          # [b, n, 128]
    zd = einsum('nm,bmh->bnh', adj, z)           # graph diffusion
    GRU(zd, h) -> h                              # gated update
Readout: y = h @ out_w.T + out_b                 # [b, n, 714]

Layout on each core (batch shard of 8):
  - token axis = b*768 + n  (n padded 714->768), TOKP = 6144 tokens
  - state kept feature-major in SBUF: h[128 hid, TOKP] bf16
  - matmuls in bf16 with fp32 PSUM accumulation
  - diffusion needs z token-major; produced via batched xbar DMA
    transposes (adjT rows >= 714 are zero, so padded-token values
    never contribute)
  - program is emitted cross-phase interleaved (GRU(t) with fc(t+1),
    GRU(5) with readout) to keep TensorE dense and HAM-warm
"""
import sys
import types

sys.path.insert(0, "/opt/trn_rl_repo")

import numpy as np
import ml_dtypes
from contextlib import ExitStack

# NTFF profile hook shim: the agent image lacks antenv.axon_hooks; provide it
# so run_bass_kernel_spmd(trace=True) can profile. Harmless when unused.
try:
    import antenv.axon_hooks  # noqa: F401
except ImportError:
    try:
        import trn_agent_boot.trn_boot as _tb

        _m = types.ModuleType("antenv.axon_hooks")
        _hook = _tb._ntff_profile_via_ctypes("/opt/axon/libaxon_pjrt.so")
        _m.get_axon_ntff_profile_hook = lambda: _hook
        _m.set_axon_ntff_profile_hook = lambda h: None
        sys.modules["antenv.axon_hooks"] = _m
    except Exception:
        pass

from concourse import bacc, tile, mybir
from concourse.bass_utils import run_bass_kernel_spmd

F32 = mybir.dt.float32
BF16 = mybir.dt.bfloat16
AF = mybir.ActivationFunctionType
ALU = mybir.AluOpType

B, T, N, D, HID = 64, 6, 714, 16, 128
CORES = 8
BL = B // CORES            # batch per core
NP = 768                   # padded graph size (6*128)
TOKP = BL * NP             # 6144 padded tokens per core
FBLK = 512                 # token block for fc/GRU matmul streams
NBLK = TOKP // FBLK        # 12
NPAIR = NBLK // 2          # 6 pairs of blocks
NCH = NP // 128            # 6 m-chunks per batch item
NH = 357                   # half of the 714 output columns
TGRP = 1536                # tokens per transpose group (2 batch items)

_NC_CACHE = {}


def _build_program():
    if "nc" in _NC_CACHE:
        return _NC_CACHE["nc"]

    nc = bacc.Bacc(
        "TRN2",
        target_bir_lowering=False,
        debug=False,
        enable_asserts=True,
        num_devices=CORES,
    )

    xT_d = nc.declare_dram_parameter("xT", [T, D, TOKP], BF16, isOutput=False)
    adjT_d = nc.declare_dram_parameter("adjT", [NP, N], BF16, isOutput=False)
    fcwx_d = nc.declare_dram_parameter("fc_wxT", [D, HID], BF16, isOutput=False)
    fcwh_d = nc.declare_dram_parameter("fc_whT", [HID, HID], BF16, isOutput=False)
    wih_d = nc.declare_dram_parameter("w_ihT", [HID, 3 * HID], BF16, isOutput=False)
    whh_d = nc.declare_dram_parameter("w_hhT", [HID, 3 * HID], BF16, isOutput=False)
    fcb_d = nc.declare_dram_parameter("fc_b", [HID, 1], F32, isOutput=False)
    br_d = nc.declare_dram_parameter("b_r", [HID, 1], F32, isOutput=False)
    bzn_d = nc.declare_dram_parameter("b_zn", [HID, 1], F32, isOutput=False)
    bin_d = nc.declare_dram_parameter("b_in", [HID, 1], F32, isOutput=False)
    bhn_d = nc.declare_dram_parameter("b_hn", [HID, 1], F32, isOutput=False)
    ident_d = nc.declare_dram_parameter("ident", [128, 128], BF16, isOutput=False)
    outw_d = nc.declare_dram_parameter("out_wT", [HID, N], BF16, isOutput=False)
    outbbc_d = nc.declare_dram_parameter("out_b_bc", [128, N], F32, isOutput=False)
    outbrow_d = nc.declare_dram_parameter("out_b_row", [1, N], BF16, isOutput=False)
    y_d = nc.declare_dram_parameter("y", [BL, N, N], F32, isOutput=True)

    with tile.TileContext(nc) as tc, ExitStack() as ctx:
        cst = ctx.enter_context(tc.tile_pool(name="cst", bufs=1))
        st = ctx.enter_context(tc.tile_pool(name="st", bufs=1))
        xt_p = ctx.enter_context(tc.tile_pool(name="xt_p", bufs=2))
        gb = ctx.enter_context(tc.tile_pool(name="gb", bufs=6))
        gb2 = ctx.enter_context(tc.tile_pool(name="gb2", bufs=5))
        ysb_p = ctx.enter_context(tc.tile_pool(name="ysb", bufs=8))
        ps = ctx.enter_context(tc.tile_pool(name="ps", bufs=8, space="PSUM"))

        # warm the ACT function tables before any real dependency exists
        dummy = cst.tile([1, 16], F32, tag="dummy")
        nc.scalar.activation(dummy[:], dummy[:], AF.Sigmoid)
        nc.scalar.activation(dummy[:], dummy[:], AF.Copy)

        # ---- constants in (fc(0)-critical tensors first) ----
        fc_wxT = cst.tile([D, HID], BF16, tag="fc_wxT")
        nc.sync.dma_start(fc_wxT[:], fcwx_d[:])
        fc_whT = cst.tile([HID, HID], BF16, tag="fc_whT")
        nc.sync.dma_start(fc_whT[:], fcwh_d[:])
        fc_b = cst.tile([HID, 1], F32, tag="fc_b")
        nc.sync.dma_start(fc_b[:], fcb_d[:])
        adjT = []
        for k in range(NCH):
            a = cst.tile([128, N], BF16, tag=f"adjT{k}", name=f"adjT{k}")
            nc.scalar.dma_start(a[:], adjT_d[128 * k : 128 * (k + 1), :])
            adjT.append(a)
        w_ihT = cst.tile([HID, 3 * HID], BF16, tag="w_ihT")
        nc.scalar.dma_start(w_ihT[:], wih_d[:])
        w_hhT = cst.tile([HID, 3 * HID], BF16, tag="w_hhT")
        nc.scalar.dma_start(w_hhT[:], whh_d[:])
        ident = cst.tile([128, 128], BF16, tag="ident")
        nc.scalar.dma_start(ident[:], ident_d[:])
        out_wT = cst.tile([HID, N], BF16, tag="out_wT")
        nc.scalar.dma_start(out_wT[:], outw_d[:])
        out_b_bc = cst.tile([128, N], F32, tag="out_b_bc")
        nc.scalar.dma_start(out_b_bc[:], outbbc_d[:])
        out_b_row = cst.tile([1, N], BF16, tag="out_b_row")
        nc.scalar.dma_start(out_b_row[:], outbrow_d[:])
        ones_col = cst.tile([1, 128], BF16, tag="ones_col")
        nc.gpsimd.memset(ones_col[:], 1.0)
        b_r = cst.tile([HID, 1], F32, tag="b_r")
        nc.scalar.dma_start(b_r[:], br_d[:])
        b_zn = cst.tile([HID, 1], F32, tag="b_zn")
        nc.scalar.dma_start(b_zn[:], bzn_d[:])
        b_in = cst.tile([HID, 1], F32, tag="b_in")
        nc.scalar.dma_start(b_in[:], bin_d[:])
        b_hn = cst.tile([HID, 1], F32, tag="b_hn")
        nc.scalar.dma_start(b_hn[:], bhn_d[:])

        # ---- state ----
        h0 = st.tile([HID, TOKP], BF16, tag="h0")
        h1 = st.tile([HID, TOKP], BF16, tag="h1")
        z_fm = st.tile([HID, TOKP], BF16, tag="z_fm")
        zd0 = st.tile([HID, TOKP], BF16, tag="zd0")
        zd1 = st.tile([HID, TOKP], BF16, tag="zd1")
        zt_all = st.tile([128, BL * NCH, 128], BF16, tag="zt_all")
        # h0 must be zero (initial state); h1 is fully written before read.
        # zd only needs its pad columns (tokens 714..767 of each window)
        # zeroed once — diffusion never writes them, GRU reads them.
        nc.vector.memset(h0[:], 0.0)
        nc.gpsimd.memset(
            zd0.rearrange("p (b n) -> p b n", b=BL)[:, :, N:NP], 0.0)
        nc.gpsimd.memset(
            zd1.rearrange("p (b n) -> p b n", b=BL)[:, :, N:NP], 0.0)
        hbuf = [h0, h1]
        zdbuf = [zd0, zd1]

        def load_xt(t):
            xt = xt_p.tile([D, TOKP], BF16, tag="xt", name="xt")
            nc.sync.dma_start(xt[:], xT_d[t])
            return xt

        def fc_block(t, i, xt, hc):
            """z[:, blk] = relu(Wx@xt + Wh@h + fc_b)"""
            s0, s1 = FBLK * i, FBLK * (i + 1)
            psz = ps.tile([128, FBLK], F32, tag="blk", name="psz")
            nc.tensor.matmul(psz[:], fc_wxT[:], xt[:, s0:s1],
                             start=True, stop=False)
            nc.tensor.matmul(psz[:], fc_whT[:], hc[:, s0:s1],
                             start=False, stop=True)
            nc.scalar.activation(z_fm[:, s0:s1], psz[:], AF.Relu,
                                 bias=fc_b[:])

        def transpose_group(j):
            """xbar-transpose tokens [1536j, 1536(j+1)) of z into zt_all."""
            nc.sync.dma_start(
                zt_all[:, 12 * j : 12 * (j + 1), :],
                z_fm[:, TGRP * j : TGRP * (j + 1)],
                transpose=True)

        def diffusion_b(b, zdn):
            """zd[:, b-window] = z_b.T-chunks @ adjT  (contract over m)."""
            base = NP * b
            psa = ps.tile([128, FBLK], F32, tag="blk", name="psa")
            psb = ps.tile([128, FBLK], F32, tag="blk", name="psb")
            for k in range(NCH):
                zt = zt_all[:, NCH * b + k, :]
                nc.tensor.matmul(psa[:, 0:NH], zt, adjT[k][:, 0:NH],
                                 start=(k == 0), stop=(k == NCH - 1))
                nc.tensor.matmul(psb[:, 0:NH], zt, adjT[k][:, NH:N],
                                 start=(k == 0), stop=(k == NCH - 1))
            if b % 2 == 0:
                nc.scalar.activation(zdn[:, base : base + NH],
                                     psa[:, 0:NH], AF.Copy)
                nc.scalar.activation(zdn[:, base + NH : base + N],
                                     psb[:, 0:NH], AF.Copy)
            else:
                nc.vector.tensor_copy(zdn[:, base : base + NH], psa[:, 0:NH])
                nc.vector.tensor_copy(zdn[:, base + NH : base + N],
                                      psb[:, 0:NH])

        def gru_pair_a(p, hc, zdc):
            """GRU stage A for blocks 2p, 2p+1: r/u/hn matmuls + r/u1/t1."""
            u2 = gb2.tile([128, 2 * FBLK], BF16, tag="u2", name="u2")
            t1s = []
            for half, i in enumerate((2 * p, 2 * p + 1)):
                s0, s1 = FBLK * i, FBLK * (i + 1)
                o0, o1 = FBLK * half, FBLK * (half + 1)
                ps_hn = ps.tile([128, FBLK], F32, tag="blk", name="ps_hn")
                nc.tensor.matmul(ps_hn[:], w_hhT[:, 256:384], hc[:, s0:s1],
                                 start=True, stop=True)
                ps_r = ps.tile([128, FBLK], F32, tag="blk", name="ps_r")
                nc.tensor.matmul(ps_r[:], w_ihT[:, 0:128], zdc[:, s0:s1],
                                 start=True, stop=False)
                nc.tensor.matmul(ps_r[:], w_hhT[:, 0:128], hc[:, s0:s1],
                                 start=False, stop=True)
                ps_u = ps.tile([128, FBLK], F32, tag="blk", name="ps_u")
                nc.tensor.matmul(ps_u[:], w_ihT[:, 128:256], zdc[:, s0:s1],
                                 start=True, stop=False)
                nc.tensor.matmul(ps_u[:], w_hhT[:, 128:256], hc[:, s0:s1],
                                 start=False, stop=True)

                r = gb.tile([128, FBLK], BF16, tag="r", name="r")
                nc.scalar.activation(r[:], ps_r[:], AF.Sigmoid, bias=b_r[:])
                nc.scalar.activation(u2[:, o0:o1], ps_u[:], AF.Sigmoid,
                                     bias=b_zn[:], scale=-1.0)
                t1 = gb.tile([128, FBLK], BF16, tag="t1", name="t1")
                nc.vector.scalar_tensor_tensor(t1[:], ps_hn[:], b_hn[:], r[:],
                                               ALU.add, ALU.mult)
                t1s.append(t1)
            return u2, t1s

        def gru_pair_b(p, hc, hn, zdc, ab):
            """GRU stage B for blocks 2p, 2p+1:
            h' = h + (1-u)*(tanh(i_n + b_in + t1) - h)."""
            u2, t1s = ab
            sg2 = gb2.tile([128, 2 * FBLK], BF16, tag="sg2", name="sg2")
            for half, i in enumerate((2 * p, 2 * p + 1)):
                s0, s1 = FBLK * i, FBLK * (i + 1)
                o0, o1 = FBLK * half, FBLK * (half + 1)
                ps_in = ps.tile([128, FBLK], F32, tag="blk", name="ps_in")
                nc.tensor.matmul(ps_in[:], w_ihT[:, 256:384], zdc[:, s0:s1],
                                 start=True, stop=True)
                nc.vector.scalar_tensor_tensor(sg2[:, o0:o1], ps_in[:],
                                               b_in[:], t1s[half][:],
                                               ALU.add, ALU.add)
            s0, s1 = 2 * FBLK * p, 2 * FBLK * (p + 1)
            c2 = gb2.tile([128, 2 * FBLK], BF16, tag="c2", name="c2")
            nc.scalar.activation(c2[:], sg2[:], AF.Tanh)
            d2 = gb2.tile([128, 2 * FBLK], BF16, tag="d2", name="d2")
            nc.vector.tensor_tensor(d2[:], c2[:], hc[:, s0:s1], ALU.subtract)
            e2 = gb2.tile([128, 2 * FBLK], BF16, tag="e2", name="e2")
            nc.vector.tensor_tensor(e2[:], u2[:], d2[:], ALU.mult)
            nc.vector.tensor_tensor(hn[:, s0:s1], hc[:, s0:s1], e2[:], ALU.add)

        def readout_chunk(c, hF):
            b, k = divmod(c, NCH)
            rows = 128 if k < NCH - 1 else N - 128 * (NCH - 1)
            tk0 = NP * b + 128 * k
            hch = hF[:, tk0:tk0 + 128]
            psa = ps.tile([128, FBLK], F32, tag="blk", name="pya")
            psb = ps.tile([128, FBLK], F32, tag="blk", name="pyb")
            y_sb = ysb_p.tile([128, N], F32, tag="y_sb", name="y_sb")
            if c % 2 == 0:
                # bias via DVE scalar_tensor_tensor; store from sync queue
                nc.tensor.matmul(psa[:, 0:NH], hch, out_wT[:, 0:NH],
                                 start=True, stop=True)
                nc.tensor.matmul(psb[:, 0:NH], hch, out_wT[:, NH:N],
                                 start=True, stop=True)
                nc.vector.scalar_tensor_tensor(
                    y_sb[:, 0:NH], psa[:, 0:NH], 0.0, out_b_bc[:, 0:NH],
                    ALU.bypass, ALU.add)
                nc.vector.scalar_tensor_tensor(
                    y_sb[:, NH:N], psb[:, 0:NH], 0.0, out_b_bc[:, NH:N],
                    ALU.bypass, ALU.add)
                nc.sync.dma_start(y_d[b, 128 * k : 128 * k + rows, :],
                                  y_sb[0:rows, :])
            else:
                # bias via rank-1 matmul + ACT copies; store from the scalar
                # queue right behind its own copies (tiny queue wait)
                nc.tensor.matmul(psa[:, 0:NH], ones_col[:],
                                 out_b_row[:, 0:NH], start=True, stop=False)
                nc.tensor.matmul(psa[:, 0:NH], hch, out_wT[:, 0:NH],
                                 start=False, stop=True)
                nc.tensor.matmul(psb[:, 0:NH], ones_col[:],
                                 out_b_row[:, NH:N], start=True, stop=False)
                nc.tensor.matmul(psb[:, 0:NH], hch, out_wT[:, NH:N],
                                 start=False, stop=True)
                nc.scalar.activation(y_sb[:, 0:NH], psa[:, 0:NH], AF.Copy)
                nc.scalar.activation(y_sb[:, NH:N], psb[:, 0:NH], AF.Copy)
                nc.scalar.dma_start(y_d[b, 128 * k : 128 * k + rows, :],
                                    y_sb[0:rows, :])

        # ---- prologue: step 0 fc + transposes + diffusion ----
        xt_cur = load_xt(0)
        for i in range(NBLK):
            fc_block(0, i, xt_cur, hbuf[0])
        for j in range(4):
            transpose_group(j)
        for b in range(BL):
            diffusion_b(b, zdbuf[0])

        # ---- main loop ----
        # GRU(t) pairs are the backbone; work for step t+1 (fc, transposes,
        # diffusion b0..b3) rides 2+ pairs behind its dependencies, and
        # diffusion(t) b4..b7 was deferred into this step's own pair loop
        # (slack >= 2 pairs everywhere so TensorE never head-of-line blocks).
        for t in range(T):
            hc, hn = hbuf[t % 2], hbuf[(t + 1) % 2]
            zdc = zdbuf[t % 2]
            zdn = zdbuf[(t + 1) % 2]
            if t + 1 < T:
                xt_nxt = load_xt(t + 1)
            ab = [None] * NPAIR
            for p in range(NPAIR):
                ab[p] = gru_pair_a(p, hc, zdc)
                if p >= 1:
                    gru_pair_b(p - 1, hc, hn, zdc, ab[p - 1])
                if t > 0 and p == 0:
                    diffusion_b(4, zdc)
                    diffusion_b(5, zdc)
                if t > 0 and p == 1:
                    diffusion_b(6, zdc)
                    diffusion_b(7, zdc)
                if t + 1 < T:
                    if p >= 2:
                        fc_block(t + 1, 2 * (p - 2), xt_nxt, hn)
                        fc_block(t + 1, 2 * (p - 2) + 1, xt_nxt, hn)
                    if p == 3:
                        transpose_group(0)
                    if p == 4:
                        transpose_group(1)
                    if p == 5:
                        fc_block(t + 1, 6, xt_nxt, hn)
                        fc_block(t + 1, 7, xt_nxt, hn)
                        diffusion_b(0, zdn)
                        diffusion_b(1, zdn)
                else:
                    if p >= 2:
                        for c in range(8 * (p - 2), 8 * (p - 1)):
                            readout_chunk(c, hn)
                    if p == NPAIR - 1:
                        for c in range(32, 40):
                            readout_chunk(c, hn)
            if t + 1 < T:
                for i in (8, 9):
                    fc_block(t + 1, i, xt_nxt, hn)
                transpose_group(2)
                gru_pair_b(NPAIR - 1, hc, hn, zdc, ab[NPAIR - 1])
                diffusion_b(2, zdn)
                diffusion_b(3, zdn)
                for i in (10, 11):
                    fc_block(t + 1, i, xt_nxt, hn)
                transpose_group(3)
            else:
                gru_pair_b(NPAIR - 1, hc, hn, zdc, ab[NPAIR - 1])
                for c in range(40, 48):
                    readout_chunk(c, hn)
            xt_cur = xt_nxt if t + 1 < T else None

    nc.compile()
    _NC_CACHE["nc"] = nc
    return nc


def _prep_core_inputs(x_core, shared):
    m = dict(shared)
    xp = np.zeros((T, D, BL, NP), dtype=np.float32)
    xp[:, :, :, :N] = x_core.transpose(1, 3, 0, 2)
    m["xT"] = xp.reshape(T, D, TOKP).astype(ml_dtypes.bfloat16)
    return m


def run(inputs, trace=False):
    x = np.asarray(inputs["x"], np.float32)
    adj = np.asarray(inputs["adj"], np.float32)
    fc_w = np.asarray(inputs["fc_w"], np.float32)
    fc_b = np.asarray(inputs["fc_b"], np.float32)
    w_ih = np.asarray(inputs["w_ih"], np.float32)
    w_hh = np.asarray(inputs["w_hh"], np.float32)
    b_ih = np.asarray(inputs["b_ih"], np.float32)
    b_hh = np.asarray(inputs["b_hh"], np.float32)
    out_w = np.asarray(inputs["out_w"], np.float32)
    out_b = np.asarray(inputs["out_b"], np.float32)

    adjT = np.zeros((NP, N), np.float32)
    adjT[:N, :] = adj.T
    shared = {
        "adjT": adjT.astype(ml_dtypes.bfloat16),
        "fc_wxT": np.ascontiguousarray(fc_w[:, :D].T).astype(ml_dtypes.bfloat16),
        "fc_whT": np.ascontiguousarray(fc_w[:, D:].T).astype(ml_dtypes.bfloat16),
        "w_ihT": np.ascontiguousarray(w_ih.T).astype(ml_dtypes.bfloat16),
        "w_hhT": np.ascontiguousarray(w_hh.T).astype(ml_dtypes.bfloat16),
        "fc_b": fc_b.reshape(HID, 1).copy(),
        "b_r": (b_ih[0:128] + b_hh[0:128]).reshape(HID, 1),
        "b_zn": (-(b_ih[128:256] + b_hh[128:256])).reshape(HID, 1),
        "b_in": b_ih[256:384].reshape(HID, 1).copy(),
        "b_hn": b_hh[256:384].reshape(HID, 1).copy(),
        "ident": np.eye(128, dtype=np.float32).astype(ml_dtypes.bfloat16),
        "out_wT": np.ascontiguousarray(out_w.T).astype(ml_dtypes.bfloat16),
        "out_b_bc": np.ascontiguousarray(
            np.broadcast_to(out_b, (128, N))).astype(np.float32),
        "out_b_row": out_b.reshape(1, N).astype(ml_dtypes.bfloat16),
    }

    nc = _build_program()
    in_maps = [_prep_core_inputs(x[BL * i : BL * (i + 1)], shared)
               for i in range(CORES)]
    res = run_bass_kernel_spmd(nc, in_maps, list(range(CORES)), trace=trace)
    y = np.concatenate([res.results[i]["y"] for i in range(CORES)], axis=0)
    return y, res


def kernel(**inputs) -> np.ndarray:
    y, _ = run(inputs, trace=False)
    return y


# revision 24
# speedup vs baseline: 1.0581x; 1.0061x over previous
"""DCRNN kernel for Trainium2 (8 NeuronCores, data-parallel over batch).

Model (per time step t, 6 steps):
    z  = relu([x_t, h] @ fc_w.T + fc_b)          # [b, n, 128]
    zd = einsum('nm,bmh->bnh', adj, z)           # graph diffusion
    GRU(zd, h) -> h                              # gated update
Readout: y = h @ out_w.T + out_b                 # [b, n, 714]

Layout on each core (batch shard of 8):
  - token axis = b*768 + n  (n padded 714->768), TOKP = 6144 tokens
  - state kept feature-major in SBUF: h[128 hid, TOKP] bf16
  - matmuls in bf16 with fp32 PSUM accumulation
  - diffusion needs z token-major; produced via batched xbar DMA
    transposes (adjT rows >= 714 are zero, so padded-token values
    never contribute)
  - program is emitted cross-phase interleaved (GRU(t) with fc(t+1),
    GRU(5) with readout) to keep TensorE dense and HAM-warm
"""
import sys
import types

sys.path.insert(0, "/opt/trn_rl_repo")

import numpy as np
import ml_dtypes
from contextlib import ExitStack

# NTFF profile hook shim: the agent image lacks antenv.axon_hooks; provide it
# so run_bass_kernel_spmd(trace=True) can profile. Harmless when unused.
try:
    import antenv.axon_hooks  # noqa: F401
except ImportError:
    try:
        import trn_agent_boot.trn_boot as _tb

        _m = types.ModuleType("antenv.axon_hooks")
        _hook = _tb._ntff_profile_via_ctypes("/opt/axon/libaxon_pjrt.so")
        _m.get_axon_ntff_profile_hook = lambda: _hook
        _m.set_axon_ntff_profile_hook = lambda h: None
        sys.modules["antenv.axon_hooks"] = _m
    except Exception:
        pass

from concourse import bacc, tile, mybir
from concourse.bass_utils import run_bass_kernel_spmd

F32 = mybir.dt.float32
BF16 = mybir.dt.bfloat16
AF = mybir.ActivationFunctionType
ALU = mybir.AluOpType

B, T, N, D, HID = 64, 6, 714, 16, 128
CORES = 8
BL = B // CORES            # batch per core
NP = 768                   # padded graph size (6*128)
TOKP = BL * NP             # 6144 padded tokens per core
FBLK = 512                 # token block for fc/GRU matmul streams
NBLK = TOKP // FBLK        # 12
NPAIR = NBLK // 2          # 6 pairs of blocks
NCH = NP // 128            # 6 m-chunks per batch item
NH = 357                   # half of the 714 output columns
TGRP = 1536                # tokens per transpose group (2 batch items)

_NC_CACHE = {}


def _build_program():
    if "nc" in _NC_CACHE:
        return _NC_CACHE["nc"]

    nc = bacc.Bacc(
        "TRN2",
        target_bir_lowering=False,
        debug=False,
        enable_asserts=True,
        num_devices=CORES,
    )

    xT_d = nc.declare_dram_parameter("xT", [T, D, TOKP], BF16, isOutput=False)
    adjT_d = nc.declare_dram_parameter("adjT", [NP, N], BF16, isOutput=False)
    fcwx_d = nc.declare_dram_parameter("fc_wxT", [D, HID], BF16, isOutput=False)
    fcwh_d = nc.declare_dram_parameter("fc_whT", [HID, HID], BF16, isOutput=False)
    wih_d = nc.declare_dram_parameter("w_ihT", [HID, 3 * HID], BF16, isOutput=False)
    whh_d = nc.declare_dram_parameter("w_hhT", [HID, 3 * HID], BF16, isOutput=False)
    fcb_d = nc.declare_dram_parameter("fc_b", [HID, 1], F32, isOutput=False)
    br_d = nc.declare_dram_parameter("b_r", [HID, 1], F32, isOutput=False)
    bzn_d = nc.declare_dram_parameter("b_zn", [HID, 1], F32, isOutput=False)
    bin_d = nc.declare_dram_parameter("b_in", [HID, 1], F32, isOutput=False)
    bhn_d = nc.declare_dram_parameter("b_hn", [HID, 1], F32, isOutput=False)
    ident_d = nc.declare_dram_parameter("ident", [128, 128], BF16, isOutput=False)
    outw_d = nc.declare_dram_parameter("out_wT", [HID, N], BF16, isOutput=False)
    outbbc_d = nc.declare_dram_parameter("out_b_bc", [128, N], F32, isOutput=False)
    outbrow_d = nc.declare_dram_parameter("out_b_row", [1, N], BF16, isOutput=False)
    y_d = nc.declare_dram_parameter("y", [BL, N, N], F32, isOutput=True)

    with tile.TileContext(nc) as tc, ExitStack() as ctx:
        cst = ctx.enter_context(tc.tile_pool(name="cst", bufs=1))
        st = ctx.enter_context(tc.tile_pool(name="st", bufs=1))
        xt_p = ctx.enter_context(tc.tile_pool(name="xt_p", bufs=2))
        gb = ctx.enter_context(tc.tile_pool(name="gb", bufs=6))
        gb2 = ctx.enter_context(tc.tile_pool(name="gb2", bufs=5))
        ysb_p = ctx.enter_context(tc.tile_pool(name="ysb", bufs=8))
        ps = ctx.enter_context(tc.tile_pool(name="ps", bufs=8, space="PSUM"))

        # warm the ACT function tables before any real dependency exists
        dummy = cst.tile([1, 16], F32, tag="dummy")
        nc.scalar.activation(dummy[:], dummy[:], AF.Sigmoid)
        nc.scalar.activation(dummy[:], dummy[:], AF.Copy)

        # ---- constants in (fc(0)-critical tensors first) ----
        xt0 = xt_p.tile([D, TOKP], BF16, tag="xt", name="xt")
        nc.sync.dma_start(xt0[:], xT_d[0])
        fc_wxT = cst.tile([D, HID], BF16, tag="fc_wxT")
        nc.sync.dma_start(fc_wxT[:], fcwx_d[:])
        fc_whT = cst.tile([HID, HID], BF16, tag="fc_whT")
        nc.sync.dma_start(fc_whT[:], fcwh_d[:])
        fc_b = cst.tile([HID, 1], F32, tag="fc_b")
        nc.sync.dma_start(fc_b[:], fcb_d[:])
        adjT = []
        for k in range(NCH):
            a = cst.tile([128, N], BF16, tag=f"adjT{k}", name=f"adjT{k}")
            nc.scalar.dma_start(a[:], adjT_d[128 * k : 128 * (k + 1), :])
            adjT.append(a)
        w_ihT = cst.tile([HID, 3 * HID], BF16, tag="w_ihT")
        nc.scalar.dma_start(w_ihT[:], wih_d[:])
        w_hhT = cst.tile([HID, 3 * HID], BF16, tag="w_hhT")
        nc.scalar.dma_start(w_hhT[:], whh_d[:])
        ident = cst.tile([128, 128], BF16, tag="ident")
        nc.scalar.dma_start(ident[:], ident_d[:])
        out_wT = cst.tile([HID, N], BF16, tag="out_wT")
        nc.scalar.dma_start(out_wT[:], outw_d[:])
        out_b_bc = cst.tile([128, N], F32, tag="out_b_bc")
        nc.scalar.dma_start(out_b_bc[:], outbbc_d[:])
        out_b_row = cst.tile([1, N], BF16, tag="out_b_row")
        nc.scalar.dma_start(out_b_row[:], outbrow_d[:])
        ones_col = cst.tile([1, 128], BF16, tag="ones_col")
        nc.gpsimd.memset(ones_col[:], 1.0)
        b_r = cst.tile([HID, 1], F32, tag="b_r")
        nc.scalar.dma_start(b_r[:], br_d[:])
        b_zn = cst.tile([HID, 1], F32, tag="b_zn")
        nc.scalar.dma_start(b_zn[:], bzn_d[:])
        b_in = cst.tile([HID, 1], F32, tag="b_in")
        nc.scalar.dma_start(b_in[:], bin_d[:])
        b_hn = cst.tile([HID, 1], F32, tag="b_hn")
        nc.scalar.dma_start(b_hn[:], bhn_d[:])

        # ---- state ----
        h0 = st.tile([HID, TOKP], BF16, tag="h0")
        h1 = st.tile([HID, TOKP], BF16, tag="h1")
        z_fm = st.tile([HID, TOKP], BF16, tag="z_fm")
        zd0 = st.tile([HID, TOKP], BF16, tag="zd0")
        zd1 = st.tile([HID, TOKP], BF16, tag="zd1")
        zt_all = st.tile([128, BL * NCH, 128], BF16, tag="zt_all")
        # h0 must be zero (initial state); h1 is fully written before read.
        # zd only needs its pad columns (tokens 714..767 of each window)
        # zeroed once — diffusion never writes them, GRU reads them.
        nc.vector.memset(h0[:], 0.0)
        nc.gpsimd.memset(
            zd0.rearrange("p (b n) -> p b n", b=BL)[:, :, N:NP], 0.0)
        nc.gpsimd.memset(
            zd1.rearrange("p (b n) -> p b n", b=BL)[:, :, N:NP], 0.0)
        hbuf = [h0, h1]
        zdbuf = [zd0, zd1]

        def load_xt(t):
            xt = xt_p.tile([D, TOKP], BF16, tag="xt", name="xt")
            nc.sync.dma_start(xt[:], xT_d[t])
            return xt

        def fc_block(t, i, xt, hc):
            """z[:, blk] = relu(Wx@xt + Wh@h + fc_b)"""
            s0, s1 = FBLK * i, FBLK * (i + 1)
            psz = ps.tile([128, FBLK], F32, tag="blk", name="psz")
            nc.tensor.matmul(psz[:], fc_wxT[:], xt[:, s0:s1],
                             start=True, stop=False)
            nc.tensor.matmul(psz[:], fc_whT[:], hc[:, s0:s1],
                             start=False, stop=True)
            nc.scalar.activation(z_fm[:, s0:s1], psz[:], AF.Relu,
                                 bias=fc_b[:])

        def transpose_group(j):
            """xbar-transpose tokens [1536j, 1536(j+1)) of z into zt_all."""
            nc.sync.dma_start(
                zt_all[:, 12 * j : 12 * (j + 1), :],
                z_fm[:, TGRP * j : TGRP * (j + 1)],
                transpose=True)

        def diffusion_b(b, zdn):
            """zd[:, b-window] = z_b.T-chunks @ adjT  (contract over m)."""
            base = NP * b
            psa = ps.tile([128, FBLK], F32, tag="blk", name="psa")
            psb = ps.tile([128, FBLK], F32, tag="blk", name="psb")
            for k in range(NCH):
                zt = zt_all[:, NCH * b + k, :]
                nc.tensor.matmul(psa[:, 0:NH], zt, adjT[k][:, 0:NH],
                                 start=(k == 0), stop=(k == NCH - 1))
                nc.tensor.matmul(psb[:, 0:NH], zt, adjT[k][:, NH:N],
                                 start=(k == 0), stop=(k == NCH - 1))
            if b % 2 == 0:
                nc.scalar.activation(zdn[:, base : base + NH],
                                     psa[:, 0:NH], AF.Copy)
                nc.scalar.activation(zdn[:, base + NH : base + N],
                                     psb[:, 0:NH], AF.Copy)
            else:
                nc.vector.tensor_copy(zdn[:, base : base + NH], psa[:, 0:NH])
                nc.vector.tensor_copy(zdn[:, base + NH : base + N],
                                      psb[:, 0:NH])

        def gru_pair_a(p, hc, zdc):
            """GRU stage A for blocks 2p, 2p+1: r/u/hn matmuls + r/u1/t1."""
            u2 = gb2.tile([128, 2 * FBLK], BF16, tag="u2", name="u2")
            t1s = []
            for half, i in enumerate((2 * p, 2 * p + 1)):
                s0, s1 = FBLK * i, FBLK * (i + 1)
                o0, o1 = FBLK * half, FBLK * (half + 1)
                ps_hn = ps.tile([128, FBLK], F32, tag="blk", name="ps_hn")
                nc.tensor.matmul(ps_hn[:], w_hhT[:, 256:384], hc[:, s0:s1],
                                 start=True, stop=True)
                ps_r = ps.tile([128, FBLK], F32, tag="blk", name="ps_r")
                nc.tensor.matmul(ps_r[:], w_ihT[:, 0:128], zdc[:, s0:s1],
                                 start=True, stop=False)
                nc.tensor.matmul(ps_r[:], w_hhT[:, 0:128], hc[:, s0:s1],
                                 start=False, stop=True)
                ps_u = ps.tile([128, FBLK], F32, tag="blk", name="ps_u")
                nc.tensor.matmul(ps_u[:], w_ihT[:, 128:256], zdc[:, s0:s1],
                                 start=True, stop=False)
                nc.tensor.matmul(ps_u[:], w_hhT[:, 128:256], hc[:, s0:s1],
                                 start=False, stop=True)

                r = gb.tile([128, FBLK], BF16, tag="r", name="r")
                nc.scalar.activation(r[:], ps_r[:], AF.Sigmoid, bias=b_r[:])
                nc.scalar.activation(u2[:, o0:o1], ps_u[:], AF.Sigmoid,
                                     bias=b_zn[:], scale=-1.0)
                t1 = gb.tile([128, FBLK], BF16, tag="t1", name="t1")
                nc.vector.scalar_tensor_tensor(t1[:], ps_hn[:], b_hn[:], r[:],
                                               ALU.add, ALU.mult)
                t1s.append(t1)
            return u2, t1s

        def gru_pair_b(p, hc, hn, zdc, ab):
            """GRU stage B for blocks 2p, 2p+1:
            h' = h + (1-u)*(tanh(i_n + b_in + t1) - h)."""
            u2, t1s = ab
            sg2 = gb2.tile([128, 2 * FBLK], BF16, tag="sg2", name="sg2")
            for half, i in enumerate((2 * p, 2 * p + 1)):
                s0, s1 = FBLK * i, FBLK * (i + 1)
                o0, o1 = FBLK * half, FBLK * (half + 1)
                ps_in = ps.tile([128, FBLK], F32, tag="blk", name="ps_in")
                nc.tensor.matmul(ps_in[:], w_ihT[:, 256:384], zdc[:, s0:s1],
                                 start=True, stop=True)
                nc.vector.scalar_tensor_tensor(sg2[:, o0:o1], ps_in[:],
                                               b_in[:], t1s[half][:],
                                               ALU.add, ALU.add)
            s0, s1 = 2 * FBLK * p, 2 * FBLK * (p + 1)
            c2 = gb2.tile([128, 2 * FBLK], BF16, tag="c2", name="c2")
            nc.scalar.activation(c2[:], sg2[:], AF.Tanh)
            d2 = gb2.tile([128, 2 * FBLK], BF16, tag="d2", name="d2")
            nc.vector.tensor_tensor(d2[:], c2[:], hc[:, s0:s1], ALU.subtract)
            e2 = gb2.tile([128, 2 * FBLK], BF16, tag="e2", name="e2")
            nc.vector.tensor_tensor(e2[:], u2[:], d2[:], ALU.mult)
            nc.vector.tensor_tensor(hn[:, s0:s1], hc[:, s0:s1], e2[:], ALU.add)

        def readout_chunk(c, hF):
            b, k = divmod(c, NCH)
            rows = 128 if k < NCH - 1 else N - 128 * (NCH - 1)
            tk0 = NP * b + 128 * k
            hch = hF[:, tk0:tk0 + 128]
            psa = ps.tile([128, FBLK], F32, tag="blk", name="pya")
            psb = ps.tile([128, FBLK], F32, tag="blk", name="pyb")
            y_sb = ysb_p.tile([128, N], F32, tag="y_sb", name="y_sb")
            if c % 2 == 0:
                # bias via DVE scalar_tensor_tensor; store from sync queue
                nc.tensor.matmul(psa[:, 0:NH], hch, out_wT[:, 0:NH],
                                 start=True, stop=True)
                nc.tensor.matmul(psb[:, 0:NH], hch, out_wT[:, NH:N],
                                 start=True, stop=True)
                nc.vector.scalar_tensor_tensor(
                    y_sb[:, 0:NH], psa[:, 0:NH], 0.0, out_b_bc[:, 0:NH],
                    ALU.bypass, ALU.add)
                nc.vector.scalar_tensor_tensor(
                    y_sb[:, NH:N], psb[:, 0:NH], 0.0, out_b_bc[:, NH:N],
                    ALU.bypass, ALU.add)
                nc.sync.dma_start(y_d[b, 128 * k : 128 * k + rows, :],
                                  y_sb[0:rows, :])
            else:
                # bias via rank-1 matmul + ACT copies; store from the scalar
                # queue right behind its own copies (tiny queue wait)
                nc.tensor.matmul(psa[:, 0:NH], ones_col[:],
                                 out_b_row[:, 0:NH], start=True, stop=False)
                nc.tensor.matmul(psa[:, 0:NH], hch, out_wT[:, 0:NH],
                                 start=False, stop=True)
                nc.tensor.matmul(psb[:, 0:NH], ones_col[:],
                                 out_b_row[:, NH:N], start=True, stop=False)
                nc.tensor.matmul(psb[:, 0:NH], hch, out_wT[:, NH:N],
                                 start=False, stop=True)
                nc.scalar.activation(y_sb[:, 0:NH], psa[:, 0:NH], AF.Copy)
                nc.scalar.activation(y_sb[:, NH:N], psb[:, 0:NH], AF.Copy)
                nc.scalar.dma_start(y_d[b, 128 * k : 128 * k + rows, :],
                                    y_sb[0:rows, :])

        # ---- prologue: step 0 fc + transposes + diffusion ----
        # warm the PE clock (HAM) with cheap matmuls while DMAs land
        ps_w = ps.tile([128, 64], F32, tag="blk", name="ps_w")
        for _ in range(44):
            nc.tensor.matmul(ps_w[:], fc_wxT[:], xt0[:, 0:64],
                             start=True, stop=True)
        xt_cur = xt0
        for i in range(NBLK):
            fc_block(0, i, xt_cur, hbuf[0])
        for j in range(4):
            transpose_group(j)
        for b in range(BL):
            diffusion_b(b, zdbuf[0])

        # ---- main loop ----
        # GRU(t) pairs are the backbone; work for step t+1 (fc, transposes,
        # diffusion b0..b3) rides 2+ pairs behind its dependencies, and
        # diffusion(t) b4..b7 was deferred into this step's own pair loop
        # (slack >= 2 pairs everywhere so TensorE never head-of-line blocks).
        for t in range(T):
            hc, hn = hbuf[t % 2], hbuf[(t + 1) % 2]
            zdc = zdbuf[t % 2]
            zdn = zdbuf[(t + 1) % 2]
            if t + 1 < T:
                xt_nxt = load_xt(t + 1)
            ab = [None] * NPAIR
            for p in range(NPAIR):
                ab[p] = gru_pair_a(p, hc, zdc)
                if p >= 1:
                    gru_pair_b(p - 1, hc, hn, zdc, ab[p - 1])
                if t > 0 and p == 0:
                    diffusion_b(4, zdc)
                    diffusion_b(5, zdc)
                if t > 0 and p == 1:
                    diffusion_b(6, zdc)
                    diffusion_b(7, zdc)
                if t + 1 < T:
                    if p >= 2:
                        fc_block(t + 1, 2 * (p - 2), xt_nxt, hn)
                        fc_block(t + 1, 2 * (p - 2) + 1, xt_nxt, hn)
                    if p == 3:
                        transpose_group(0)
                    if p == 4:
                        transpose_group(1)
                    if p == 5:
                        fc_block(t + 1, 6, xt_nxt, hn)
                        fc_block(t + 1, 7, xt_nxt, hn)
                        diffusion_b(0, zdn)
                        diffusion_b(1, zdn)
                else:
                    if p >= 2:
                        for c in range(8 * (p - 2), 8 * (p - 1)):
                            readout_chunk(c, hn)
                    if p == NPAIR - 1:
                        for c in range(32, 40):
                            readout_chunk(c, hn)
            if t + 1 < T:
                for i in (8, 9):
                    fc_block(t + 1, i, xt_nxt, hn)
                transpose_group(2)
                gru_pair_b(NPAIR - 1, hc, hn, zdc, ab[NPAIR - 1])
                diffusion_b(2, zdn)
                diffusion_b(3, zdn)
                for i in (10, 11):
                    fc_block(t + 1, i, xt_nxt, hn)
                transpose_group(3)
            else:
                gru_pair_b(NPAIR - 1, hc, hn, zdc, ab[NPAIR - 1])
                for c in range(40, 48):
                    readout_chunk(c, hn)
            xt_cur = xt_nxt if t + 1 < T else None

    nc.compile()
    _NC_CACHE["nc"] = nc
    return nc


def _prep_core_inputs(x_core, shared):
    m = dict(shared)
    xp = np.zeros((T, D, BL, NP), dtype=np.float32)
    xp[:, :, :, :N] = x_core.transpose(1, 3, 0, 2)
    m["xT"] = xp.reshape(T, D, TOKP).astype(ml_dtypes.bfloat16)
    return m


def run(inputs, trace=False):
    x = np.asarray(inputs["x"], np.float32)
    adj = np.asarray(inputs["adj"], np.float32)
    fc_w = np.asarray(inputs["fc_w"], np.float32)
    fc_b = np.asarray(inputs["fc_b"], np.float32)
    w_ih = np.asarray(inputs["w_ih"], np.float32)
    w_hh = np.asarray(inputs["w_hh"], np.float32)
    b_ih = np.asarray(inputs["b_ih"], np.float32)
    b_hh = np.asarray(inputs["b_hh"], np.float32)
    out_w = np.asarray(inputs["out_w"], np.float32)
    out_b = np.asarray(inputs["out_b"], np.float32)

    adjT = np.zeros((NP, N), np.float32)
    adjT[:N, :] = adj.T
    shared = {
        "adjT": adjT.astype(ml_dtypes.bfloat16),
        "fc_wxT": np.ascontiguousarray(fc_w[:, :D].T).astype(ml_dtypes.bfloat16),
        "fc_whT": np.ascontiguousarray(fc_w[:, D:].T).astype(ml_dtypes.bfloat16),
        "w_ihT": np.ascontiguousarray(w_ih.T).astype(ml_dtypes.bfloat16),
        "w_hhT": np.ascontiguousarray(w_hh.T).astype(ml_dtypes.bfloat16),
        "fc_b": fc_b.reshape(HID, 1).copy(),
        "b_r": (b_ih[0:128] + b_hh[0:128]).reshape(HID, 1),
        "b_zn": (-(b_ih[128:256] + b_hh[128:256])).reshape(HID, 1),
        "b_in": b_ih[256:384].reshape(HID, 1).copy(),
        "b_hn": b_hh[256:384].reshape(HID, 1).copy(),
        "ident": np.eye(128, dtype=np.float32).astype(ml_dtypes.bfloat16),
        "out_wT": np.ascontiguousarray(out_w.T).astype(ml_dtypes.bfloat16),
        "out_b_bc": np.ascontiguousarray(
            np.broadcast_to(out_b, (128, N))).astype(np.float32),
        "out_b_row": out_b.reshape(1, N).astype(ml_dtypes.bfloat16),
    }

    nc = _build_program()
    in_maps = [_prep_core_inputs(x[BL * i : BL * (i + 1)], shared)
               for i in range(CORES)]
    res = run_bass_kernel_spmd(nc, in_maps, list(range(CORES)), trace=trace)
    y = np.concatenate([res.results[i]["y"] for i in range(CORES)], axis=0)
    return y, res


def kernel(**inputs) -> np.ndarray:
    y, _ = run(inputs, trace=False)
    return y
